# revision 6
# baseline (speedup 1.0000x reference)
"""Joint Maximum Mean Discrepancy loss on 8 Trainium2 NeuronCores.

Math: for streams (s0,t0) and (s1,t1), the reference builds per-stream
Gaussian kernels K_r = exp(-gamma_r * dist_r) over feats_r = [src; tgt]
(N=8192 rows), takes their elementwise product, and returns
mean(s2s + t2t - 2*s2t) over the B x B blocks.

Device decomposition:
  exponent E_ij = 2*g0*(X0_i . X0_j) + 2*g1*(X1_i . X1_j) - c_i - c_j,
  c_i = g0*|X0_i|^2 + g1*|X1_i|^2, gamma_r from the closed form
  sum(dist_r) = 2*N*sum(sq_r) - 2*||colsum(X_r)||^2. The joint kernel is
  exp(E); the loss is a signed/weighted sum of exp over the 136 unordered
  512-row chunk-pair blocks (symmetry halves the N x N work).

  All contraction data is fp8 e4m3 in DoubleRow layout (2 K-rows per
  partition, 2x PE rate; tile[p,s,x] = rows[s*P+p, x]):
    - F: 256 rows sqrt(2*g0*SCALE)*X0 as [128, 2, 512]
    - G: 66 rows [sqrt(2*g1*SCALE)*X1 ; ones ; -SCALE*c] as [33, 2, 512],
      with lhs/rhs variants swapping the ones/-SCALE*c rows.
  PSUM accumulates P = SCALE*E in two DR matmuls per [128,512] m-tile
  (no PE dtype switching). ScalarE applies Exp with scale=1/SCALE into
  SBUF bf16; VectorE folds halves with tensor_add (2x bf16 mode) and
  tensor_reduce's each block to a per-partition sum column (acc [128,18]).
  fp8 end-to-end loss error vs float64 is ~3e-3 (measured host-sim),
  inside the 2e-2 budget.

Block cover (SPMD): a fixed 18-block pattern over 8 chunk "slots";
core k maps slot v to chunk (S[v] + 2k) mod 16, S = (0,1,2,3,4,5,8,9).
The 8 shifted copies tile all 120 chunk pairs + 16 loops: difference
classes d=1..7 x base-parity are hit exactly once (host weight 2),
d=8 pairs twice (weight 1), loops once (weight 1). The host applies
weight * sign (sign -1 iff exactly one chunk is a target chunk >= 8)
and reduces in float64.
"""

import os

import numpy as np

import concourse.bacc as bacc
import concourse.bass as bass
import concourse.mybir as mybir
import concourse.tile as tile
from concourse.bass_utils import run_bass_kernel_spmd

B = 4096
D0, D1 = 256, 64
N = 2 * B
CH = 512          # rows per chunk
NCHUNK = 16
NCORE = 8
MT = 128          # m-tile rows
NMT = CH // MT    # m-tiles per block row (4)
SCALE = 64.0      # exponent pre-scale; exp applies 1/SCALE
GP = 33           # partitions of the 66-row DR tile

# cyclic support: slot v of core k is chunk (S[v] + 2k) % 16
S_SUPPORT = (0, 1, 2, 3, 4, 5, 8, 9)
NSLOT = 8
# 18-block pattern in slot indices, ordered so early blocks touch early
# slots (DMA pipelining): loops (0,0),(1,1); one pair per (diff 1..7,
# parity) class; both d=8 classes.
PATTERN = [
    (0, 0), (1, 1), (0, 1),
    (1, 2), (0, 2),
    (1, 3), (0, 3),
    (1, 4), (0, 4),
    (1, 5), (0, 5),
    (3, 6), (2, 6), (1, 6), (0, 6),
    (3, 7), (2, 7), (1, 7),
]
NBLK = len(PATTERN)  # 18

F8 = mybir.dt.float8e4
BF = mybir.dt.bfloat16
F32 = mybir.dt.float32

_N_WARMUP = int(os.environ.get("JMMD_WARMUP", "24"))

LAST_EXEC_NS = None
LAST_RESULTS = None

_CACHE: dict = {}


def _build():
    if "nc" in _CACHE:
        return _CACHE["nc"]
    nc = bacc.Bacc(
        "TRN2", target_bir_lowering=False, debug=False, enable_asserts=False
    )
    f8_dram = nc.dram_tensor("f8", [MT, NSLOT, 2, CH], F8, kind="ExternalInput").ap()
    g8_dram = nc.dram_tensor("g8", [GP, NSLOT, 2, 2 * CH], F8, kind="ExternalInput").ap()
    wz_dram = nc.dram_tensor("wz", [MT, 8], BF, kind="ExternalInput").ap()
    w8_dram = nc.dram_tensor("w8", [MT, 2, MT], F8, kind="ExternalInput").ap()
    acc_dram = nc.dram_tensor("acc", [MT, NBLK], F32, kind="ExternalOutput").ap()

    with tile.TileContext(nc) as tc:
        with (
            tc.tile_pool(name="const", bufs=1) as const,
            tc.tile_pool(name="exp", bufs=2) as expp,
            tc.tile_pool(name="red", bufs=2) as redp,
            tc.tile_pool(name="psum", bufs=2, space=bass.MemorySpace.PSUM) as psum,
        ):
            fall = const.tile([MT, NSLOT, 2, CH], F8, tag="fall")
            gall = const.tile([GP, NSLOT, 2, 2 * CH], F8, tag="gall")
            wz = const.tile([MT, 8], BF, tag="wz")
            # ACT-table preload source + slot-0 operands race down the
            # scalar (HWDGE) queue; bulk slots split sync/gpsimd.
            w8 = const.tile([MT, 2, MT], F8, tag="w8")
            nc.scalar.dma_start(wz[:], wz_dram)
            nc.scalar.dma_start(w8[:], w8_dram)
            nc.scalar.dma_start(fall[:, 0:1], f8_dram[:, 0:1])
            nc.scalar.dma_start(gall[:, 0:1], g8_dram[:, 0:1])
            nc.sync.dma_start(fall[:, 1:4], f8_dram[:, 1:4])
            nc.sync.dma_start(gall[:, 1:4], g8_dram[:, 1:4])
            nc.gpsimd.dma_start(fall[:, 4:NSLOT], f8_dram[:, 4:NSLOT])
            nc.gpsimd.dma_start(gall[:, 4:NSLOT], g8_dram[:, 4:NSLOT])

            acc_t = const.tile([MT, NBLK], F32, tag="acc")

            # Exp ACT-table preload while input DMAs stream
            warm_act = const.tile([MT, 8], BF, tag="warm_act")
            nc.scalar.activation(
                warm_act[:], wz[:], mybir.ActivationFunctionType.Exp
            )

            # HAM warmup: dummy fp8-DR matmuls so real matmuls start at the
            # warm PE clock (without these the whole run stays at K=4/8).
            if _N_WARMUP:
                warm_ps = psum.tile([MT, NMT * CH], F32, tag="ps")
                for _ in range(_N_WARMUP):
                    nc.tensor.matmul(
                        warm_ps[:, :MT],
                        w8[:],
                        w8[:],
                        start=True,
                        stop=True,
                        perf_mode=mybir.MatmulPerfMode.DoubleRow,
                    )

            HF = NMT * CH // 2
            for col, (r, c) in enumerate(PATTERN):
                ps = psum.tile([MT, NMT * CH], F32, tag="ps")
                for m in range(NMT):
                    nc.tensor.matmul(
                        ps[:, m * CH:(m + 1) * CH],
                        fall[:, r, :, m * MT:(m + 1) * MT],
                        fall[:, c],
                        start=True,
                        stop=False,
                        perf_mode=mybir.MatmulPerfMode.DoubleRow,
                    )
                    nc.tensor.matmul(
                        ps[:, m * CH:(m + 1) * CH],
                        gall[:, r, :, m * MT:(m + 1) * MT],
                        gall[:, c, :, CH:],
                        start=False,
                        stop=True,
                        perf_mode=mybir.MatmulPerfMode.DoubleRow,
                    )
                ex = expp.tile([MT, NMT * CH], BF, tag="ex")
                nc.scalar.activation(
                    ex[:], ps[:], mybir.ActivationFunctionType.Exp, scale=1.0 / SCALE
                )
                red = redp.tile([MT, HF], BF, tag="red")
                nc.vector.tensor_add(red[:], ex[:, :HF], ex[:, HF:])
                red2 = redp.tile([MT, HF // 2], BF, tag="red2")
                nc.vector.tensor_add(red2[:], red[:, :HF // 2], red[:, HF // 2:])
                nc.vector.tensor_reduce(
                    acc_t[:, col:col + 1],
                    red2[:],
                    axis=mybir.AxisListType.X,
                    op=mybir.AluOpType.add,
                )
            nc.sync.dma_start(acc_dram, acc_t[:])
    nc.compile()
    _CACHE["nc"] = nc
    return nc


def _dr_pack(Wrows):
    """[2*P, X] contraction rows -> DR tile [P, 2, X] with
    tile[p, s, x] = Wrows[s*P + p, x]."""
    P = Wrows.shape[0] // 2
    return np.ascontiguousarray(
        Wrows.reshape(2, P, Wrows.shape[1]).transpose(1, 0, 2)
    )


def _pack_inputs(s0, s1, t0, t1):
    import ml_dtypes

    X0 = np.concatenate([s0, t0], axis=0).astype(np.float64)
    X1 = np.concatenate([s1, t1], axis=0).astype(np.float64)

    def gamma_of(X):
        sq = np.sum(X * X, axis=1)
        sdist = 2.0 * N * np.sum(sq) - 2.0 * np.sum(np.sum(X, axis=0) ** 2)
        return (N * N - N) / sdist, sq

    g0, sq0 = gamma_of(X0)
    g1, sq1 = gamma_of(X1)
    c = g0 * sq0 + g1 * sq1

    f8 = ml_dtypes.float8_e4m3
    W0 = np.clip(np.sqrt(2.0 * g0 * SCALE) * X0, -240, 240).astype(f8)
    W1 = np.clip(np.sqrt(2.0 * g1 * SCALE) * X1, -240, 240).astype(f8)
    cq = np.clip(-SCALE * c, -240, 240).astype(f8)

    fch, gch = [], []
    for ch in range(NCHUNK):
        rows = slice(ch * CH, (ch + 1) * CH)
        fch.append(_dr_pack(W0[rows].T))           # [128, 2, 512]
        gl = np.empty((2 * GP, CH), dtype=f8)
        gr = np.empty((2 * GP, CH), dtype=f8)
        gl[:D1] = W1[rows].T
        gr[:D1] = W1[rows].T
        gl[D1] = 1.0
        gl[D1 + 1] = cq[rows]
        gr[D1] = cq[rows]
        gr[D1 + 1] = 1.0
        g2 = np.concatenate(
            [_dr_pack(gl), _dr_pack(gr)], axis=2
        )                                          # [33, 2, 1024]
        gch.append(g2)

    wz = np.zeros((MT, 8), dtype=ml_dtypes.bfloat16)
    w8z = np.zeros((MT, 2, MT), dtype=f8)
    in_maps = []
    for k in range(NCORE):
        slots = [(S_SUPPORT[v] + 2 * k) % NCHUNK for v in range(NSLOT)]
        f8a = np.ascontiguousarray(
            np.stack([fch[ch] for ch in slots], axis=1)
        )                                          # [128, 8, 2, 512]
        g8a = np.ascontiguousarray(
            np.stack([gch[ch] for ch in slots], axis=1)
        )                                          # [33, 8, 2, 1024]
        in_maps.append({"f8": f8a, "g8": g8a, "wz": wz, "w8": w8z})
    return in_maps


def _combine(results):
    total = 0.0
    for k in range(NCORE):
        acc = np.asarray(results[k]["acc"], dtype=np.float64)  # [128, NBLK]
        colsum = acc.sum(axis=0)
        for col, (r, c) in enumerate(PATTERN):
            u = (S_SUPPORT[r] + 2 * k) % NCHUNK
            v = (S_SUPPORT[c] + 2 * k) % NCHUNK
            d = min((v - u) % NCHUNK, (u - v) % NCHUNK)
            w = 2.0 if 0 < d < 8 else 1.0        # loops and d=8 (doubled): 1
            s = (1.0 if u < 8 else -1.0) * (1.0 if v < 8 else -1.0)
            total += w * s * colsum[col]
    return total / (B * B)


def kernel(s0, s1, t0, t1):
    global LAST_EXEC_NS, LAST_RESULTS
    nc = _build()
    in_maps = _pack_inputs(
        np.asarray(s0), np.asarray(s1), np.asarray(t0), np.asarray(t1)
    )
    trace = os.environ.get("JMMD_TRACE", "0") == "1"
    res = run_bass_kernel_spmd(nc, in_maps, core_ids=list(range(NCORE)), trace=trace)
    LAST_EXEC_NS = res.exec_time_ns
    LAST_RESULTS = res
    return np.float32(_combine(res.results))


# revision 7
# speedup vs baseline: 1.9503x; 1.9503x over previous
"""Joint Maximum Mean Discrepancy loss on 8 Trainium2 NeuronCores.

Math: for streams (s0,t0) and (s1,t1), the reference builds per-stream
Gaussian kernels K_r = exp(-gamma_r * dist_r) over feats_r = [src; tgt]
(N=8192 rows), takes their elementwise product, and returns
mean(s2s + t2t - 2*s2t) over the B x B blocks.

Device decomposition:
  exponent E_ij = 2*g0*(X0_i . X0_j) + 2*g1*(X1_i . X1_j) - c_i - c_j,
  c_i = g0*|X0_i|^2 + g1*|X1_i|^2, gamma_r from the closed form
  sum(dist_r) = 2*N*sum(sq_r) - 2*||colsum(X_r)||^2. The joint kernel is
  exp(E); the loss is a signed/weighted sum of exp over the 136 unordered
  512-row chunk-pair blocks (symmetry halves the N x N work).

  PSUM accumulates P = SCALE*E from two matmuls per [128,512] m-tile:
    - fp8 e4m3 DoubleRow over the 256 stream-0 rows (2 K-rows/partition):
      rows sqrt(2*g0*SCALE)*X0, layout [128, 2, 512] with
      tile[p,s,x] = W0T[s*128+p, x]
    - bf16 over 66 rows: [sqrt(2*g1*SCALE)*X1 (64) ; ones ; -SCALE*c]
      (lhs variant) vs [... ; -SCALE*c ; ones] (rhs variant)
  ScalarE applies Exp with scale=1/SCALE into SBUF bf16 (the pace-setting
  engine: 18 x ~1.97us); VectorE folds halves twice with tensor_add
  (2x bf16 mode) then tensor_reduce's [128,512] to a per-partition sum
  column (acc [128,18], ~1.8us/block - just under ScalarE).
  fp8 end-to-end loss error vs float64 is ~2e-3 (measured host-sim),
  well inside the 2e-2 budget.

Block cover (SPMD): a fixed 18-block pattern over 8 chunk "slots";
core k maps slot v to chunk (S[v] + 2k) mod 16, S = (0,1,2,3,4,5,8,9).
The 8 shifted copies tile all 120 chunk pairs + 16 loops: difference
classes d=1..7 x base-parity are hit exactly once (host weight 2),
d=8 pairs twice (weight 1), loops once (weight 1). The host applies
weight * sign (sign -1 iff exactly one chunk is a target chunk >= 8)
and reduces in float64.
"""

import os

import numpy as np

import concourse.bacc as bacc
import concourse.bass as bass
import concourse.mybir as mybir
import concourse.tile as tile
from concourse.bass_utils import run_bass_kernel_spmd

B = 4096
D0, D1 = 256, 64
N = 2 * B
CH = 512          # rows per chunk
NCHUNK = 16
NCORE = 8
MT = 128          # m-tile rows
NMT = CH // MT    # m-tiles per block row (4)
SCALE = 64.0      # exponent pre-scale; exp applies 1/SCALE
KB = D1 + 2       # bf16 contraction rows (66)

# cyclic support: slot v of core k is chunk (S[v] + 2k) % 16
S_SUPPORT = (0, 1, 2, 3, 4, 5, 8, 9)
NSLOT = 8
# 18-block pattern in slot indices, ordered so early blocks touch early
# slots (DMA pipelining): loops (0,0),(1,1); one pair per (diff 1..7,
# parity) class; both d=8 classes.
PATTERN = [
    (0, 0), (1, 1), (0, 1),
    (1, 2), (0, 2),
    (1, 3), (0, 3),
    (1, 4), (0, 4),
    (1, 5), (0, 5),
    (3, 6), (2, 6), (1, 6), (0, 6),
    (3, 7), (2, 7), (1, 7),
]
NBLK = len(PATTERN)  # 18

F8 = mybir.dt.float8e4
BF = mybir.dt.bfloat16
F32 = mybir.dt.float32

_N_WARMUP = int(os.environ.get("JMMD_WARMUP", "36"))

LAST_EXEC_NS = None
LAST_RESULTS = None

_CACHE: dict = {}


def _build():
    if "nc" in _CACHE:
        return _CACHE["nc"]
    nc = bacc.Bacc(
        "TRN2", target_bir_lowering=False, debug=False, enable_asserts=False
    )
    f8_dram = nc.dram_tensor("f8", [NSLOT, MT, 2, CH], F8, kind="ExternalInput").ap()
    lb_dram = nc.dram_tensor("lb", [NSLOT, KB, CH], BF, kind="ExternalInput").ap()
    rb_dram = nc.dram_tensor("rb", [NSLOT, KB, CH], BF, kind="ExternalInput").ap()
    wz_dram = nc.dram_tensor("wz", [MT, 8], BF, kind="ExternalInput").ap()
    w8_dram = nc.dram_tensor("w8", [MT, 2, MT], F8, kind="ExternalInput").ap()
    acc_dram = nc.dram_tensor("acc", [MT, NBLK], F32, kind="ExternalOutput").ap()

    with tile.TileContext(nc) as tc:
        with (
            tc.tile_pool(name="const", bufs=1) as const,
            tc.tile_pool(name="exp", bufs=2) as expp,
            tc.tile_pool(name="red", bufs=2) as redp,
            tc.tile_pool(name="psum", bufs=2, space=bass.MemorySpace.PSUM) as psum,
        ):
            # warmup sources + slot-0 operands race down the scalar (HWDGE)
            # queue; the rest splits sync/gpsimd as in the proven baseline.
            wz = const.tile([MT, 8], BF, tag="wz")
            w8 = const.tile([MT, 2, MT], F8, tag="w8")
            nc.scalar.dma_start(w8[:], w8_dram)
            nc.scalar.dma_start(wz[:], wz_dram)

            ft, lt, rt = {}, {}, {}
            for j in range(NSLOT):
                ft[j] = const.tile([MT, 2, CH], F8, name=f"f{j}", tag=f"f{j}")
                lt[j] = const.tile([KB, CH], BF, name=f"l{j}", tag=f"l{j}")
                rt[j] = const.tile([KB, CH], BF, name=f"r{j}", tag=f"r{j}")
                if j == 0:
                    qa, qb = nc.scalar, nc.scalar
                else:
                    qa, qb = (nc.sync, nc.gpsimd) if j % 2 == 0 else (nc.gpsimd, nc.sync)
                qa.dma_start(ft[j][:], f8_dram[j])
                qb.dma_start(lt[j][:], lb_dram[j])
                qb.dma_start(rt[j][:], rb_dram[j])

            acc_t = const.tile([MT, NBLK], F32, tag="acc")

            # Exp ACT-table preload while input DMAs stream
            warm_act = const.tile([MT, 8], BF, tag="warm_act")
            nc.scalar.activation(
                warm_act[:], wz[:], mybir.ActivationFunctionType.Exp
            )

            # HAM warmup: dummy fp8-DR matmuls spanning > the 3.4us HAM
            # activity window so real matmuls start at the warm PE clock.
            if _N_WARMUP:
                warm_ps = psum.tile([MT, NMT * CH], F32, tag="ps")
                for _ in range(_N_WARMUP):
                    nc.tensor.matmul(
                        warm_ps[:, :MT],
                        w8[:],
                        w8[:],
                        start=True,
                        stop=True,
                        perf_mode=mybir.MatmulPerfMode.DoubleRow,
                    )

            HF = NMT * CH // 2
            for col, (r, c) in enumerate(PATTERN):
                ps = psum.tile([MT, NMT * CH], F32, tag="ps")
                for m in range(NMT):
                    nc.tensor.matmul(
                        ps[:, m * CH:(m + 1) * CH],
                        ft[r][:, :, m * MT:(m + 1) * MT],
                        ft[c][:],
                        start=True,
                        stop=False,
                        perf_mode=mybir.MatmulPerfMode.DoubleRow,
                    )
                    nc.tensor.matmul(
                        ps[:, m * CH:(m + 1) * CH],
                        lt[r][:, m * MT:(m + 1) * MT],
                        rt[c][:],
                        start=False,
                        stop=True,
                    )
                ex = expp.tile([MT, NMT * CH], BF, tag="ex")
                nc.scalar.activation(
                    ex[:], ps[:], mybir.ActivationFunctionType.Exp, scale=1.0 / SCALE
                )
                red = redp.tile([MT, HF], BF, tag="red")
                nc.vector.tensor_add(red[:], ex[:, :HF], ex[:, HF:])
                red2 = redp.tile([MT, HF // 2], BF, tag="red2")
                nc.vector.tensor_add(red2[:], red[:, :HF // 2], red[:, HF // 2:])
                nc.vector.tensor_reduce(
                    acc_t[:, col:col + 1],
                    red2[:],
                    axis=mybir.AxisListType.X,
                    op=mybir.AluOpType.add,
                )
            nc.sync.dma_start(acc_dram, acc_t[:])
    nc.compile()
    _CACHE["nc"] = nc
    return nc


def _dr_pack(Wrows):
    """[2*P, X] contraction rows -> DR tile [P, 2, X] with
    tile[p, s, x] = Wrows[s*P + p, x]."""
    P = Wrows.shape[0] // 2
    return np.ascontiguousarray(
        Wrows.reshape(2, P, Wrows.shape[1]).transpose(1, 0, 2)
    )


def _pack_inputs(s0, s1, t0, t1):
    import ml_dtypes

    X0 = np.concatenate([s0, t0], axis=0).astype(np.float64)
    X1 = np.concatenate([s1, t1], axis=0).astype(np.float64)

    def gamma_of(X):
        sq = np.sum(X * X, axis=1)
        sdist = 2.0 * N * np.sum(sq) - 2.0 * np.sum(np.sum(X, axis=0) ** 2)
        return (N * N - N) / sdist, sq

    g0, sq0 = gamma_of(X0)
    g1, sq1 = gamma_of(X1)
    c = g0 * sq0 + g1 * sq1

    f8 = ml_dtypes.float8_e4m3
    W0 = np.clip(np.sqrt(2.0 * g0 * SCALE) * X0, -240, 240).astype(f8)
    W1 = (np.sqrt(2.0 * g1 * SCALE) * X1).astype(ml_dtypes.bfloat16)
    cq = (-SCALE * c).astype(ml_dtypes.bfloat16)

    fch, lch, rch = [], [], []
    for ch in range(NCHUNK):
        rows = slice(ch * CH, (ch + 1) * CH)
        fch.append(_dr_pack(W0[rows].T))           # [128, 2, 512]
        lb = np.empty((KB, CH), dtype=ml_dtypes.bfloat16)
        rb = np.empty((KB, CH), dtype=ml_dtypes.bfloat16)
        lb[:D1] = W1[rows].T
        rb[:D1] = W1[rows].T
        lb[D1] = 1.0
        lb[D1 + 1] = cq[rows]
        rb[D1] = cq[rows]
        rb[D1 + 1] = 1.0
        lch.append(lb)
        rch.append(rb)

    wz = np.zeros((MT, 8), dtype=ml_dtypes.bfloat16)
    w8z = np.zeros((MT, 2, MT), dtype=f8)
    in_maps = []
    for k in range(NCORE):
        slots = [(S_SUPPORT[v] + 2 * k) % NCHUNK for v in range(NSLOT)]
        in_maps.append(
            {
                "f8": np.ascontiguousarray(np.stack([fch[ch] for ch in slots])),
                "lb": np.ascontiguousarray(np.stack([lch[ch] for ch in slots])),
                "rb": np.ascontiguousarray(np.stack([rch[ch] for ch in slots])),
                "wz": wz,
                "w8": w8z,
            }
        )
    return in_maps


def _combine(results):
    total = 0.0
    for k in range(NCORE):
        acc = np.asarray(results[k]["acc"], dtype=np.float64)  # [128, NBLK]
        colsum = acc.sum(axis=0)
        for col, (r, c) in enumerate(PATTERN):
            u = (S_SUPPORT[r] + 2 * k) % NCHUNK
            v = (S_SUPPORT[c] + 2 * k) % NCHUNK
            d = min((v - u) % NCHUNK, (u - v) % NCHUNK)
            w = 2.0 if 0 < d < 8 else 1.0        # loops and d=8 (doubled): 1
            s = (1.0 if u < 8 else -1.0) * (1.0 if v < 8 else -1.0)
            total += w * s * colsum[col]
    return total / (B * B)


def kernel(s0, s1, t0, t1):
    global LAST_EXEC_NS, LAST_RESULTS
    nc = _build()
    in_maps = _pack_inputs(
        np.asarray(s0), np.asarray(s1), np.asarray(t0), np.asarray(t1)
    )
    trace = os.environ.get("JMMD_TRACE", "0") == "1"
    res = run_bass_kernel_spmd(nc, in_maps, core_ids=list(range(NCORE)), trace=trace)
    LAST_EXEC_NS = res.exec_time_ns
    LAST_RESULTS = res
    return np.float32(_combine(res.results))
